# revision 22
# baseline (speedup 1.0000x reference)
"""Trainium2 kernel for nn_DWT_Features.

The reference applies a 3-level db4 DWT along the time axis of every
(batch, pixel) signal, then contracts the coefficients with a full-volume
conv kernel and applies LeakyReLU.  The DWT is a linear map sig[64] ->
coeffs[84], so the whole network collapses to a single GEMM:

    out = leaky_relu(X @ W_eff + b),  X: [B, 4096], W_eff: [4096, 64]

where W_eff[(t,h,w), k] = sum_c M[t, c] * conv_w[k, c, h, w] and M is the
64x84 DWT matrix (computed here in numpy, folded on host - O(22M) flops).

Sharding: pure data parallel, batch split across 8 cores (1024 rows each).

Device kernel design (v2):
  - X is pre-transposed AND cast to bf16 on the host into the layout
    xprep[p, kc*1024 + b] = X[b, kc*128 + p]  (p: contraction partition,
    kc: one of 32 contraction chunks, b: batch column).  This removes all
    on-chip transposes (which dominated the v1 kernel: 256 tensor-engine
    transpose matmuls + PSUM->SBUF copies) and halves HBM traffic.
  - 16 streaming DMAs of [128, 2048] bf16 (4 KiB/partition each).
  - GEMM: for each kc, two matmuls (batch blocks of 512) accumulate
    acc[64, 512] += W_kc.T @ X_kc in two PSUM banks; bf16 streams at
    1 col/cycle so the tensor engine needs ~14 us, under the ~28 us DMA
    floor -> the kernel is DMA-bound, as it should be (8.4 MB/core read).
  - bias + LeakyReLU epilogue on DVE, C.T stored; host transposes back.
"""

import sys

import numpy as np

if "/opt/trn_rl_repo" not in sys.path:
    sys.path.insert(0, "/opt/trn_rl_repo")

B, T, HW, K = 8192, 64, 8, 64
NCORES = 8
BPC = B // NCORES  # 1024 batch rows per core
F = T * HW * HW  # 4096 contracted features
NEG_SLOPE = 0.001
FILT_LEN = 8
NKC = F // 128  # 32 contraction chunks of 128
BBLK = 512  # batch columns per PSUM accumulator
NB = BPC // BBLK  # 2 batch blocks
# x DMA chunking (units of kc = BPC columns = 2 KiB/partition of bf16).
# Full-partition [128, cols] DMAs spread over the two hardware DGE queues
# (sync/scalar).  8 KiB descriptors halve the descriptor count: all dynamic
# queues are dispatched by DMA engine 79, so descriptor bookkeeping makes
# it the stream straggler — fewer descriptors shrink its lag.  Small head
# chunks let the first matmuls start early; 1-kc tail chunks shrink the
# last-chunk-landing to last-matmul gap.  W is split into 4 column chunks
# interleaved near the queue heads so the first matmul only waits for
# wa + x0, and both queues carry identical byte loads (18 kc-equivalents).
DMA_KCS = [2, 2, 4, 4, 4, 4, 4, 4, 2, 2]
X_QUEUE = [0, 1, 0, 1, 0, 1, 0, 1, 0, 1]  # 0=sync, 1=scalar
assert sum(DMA_KCS) == NKC

DB4_LO = np.array(
    [-0.010597401784997278, 0.032883011666982945, 0.030841381835986965,
     -0.18703481171888114, -0.02798376941698385, 0.6308807679295904,
     0.7148465705525415, 0.23037781330885523], dtype=np.float64)
DB4_HI = np.array(
    [-0.23037781330885523, 0.7148465705525415, -0.6308807679295904,
     -0.02798376941698385, 0.18703481171888114, 0.030841381835986965,
     0.032883011666982945, -0.010597401784997278], dtype=np.float64)


def _afb1d(x):
    # numpy mirror of the reference: reflect pad, correlate with reversed
    # filters, stride 2.  x: [N, n] float64.
    n = x.shape[-1]
    out = (n + FILT_LEN - 1) // 2
    p = 2 * (out - 1) - n + FILT_LEN
    xp = np.pad(x, ((0, 0), (p // 2, (p + 1) // 2)), mode="reflect")
    idx = 2 * np.arange(out)[:, None] + np.arange(FILT_LEN)[None, :]
    win = xp[:, idx]  # [N, out, 8]
    return win @ DB4_LO[::-1], win @ DB4_HI[::-1]


def _dwt_matrix():
    # M [64, 84] with coeffs = sig @ M (image of the identity basis).
    lo, his = np.eye(T, dtype=np.float64), []
    for _ in range(3):
        lo, hi = _afb1d(lo)
        his.append(hi)
    return np.concatenate([lo] + his, axis=-1)


def _build_bass():
    import concourse.bacc as bacc
    import concourse.mybir as mybir
    import concourse.tile as tile

    f32 = mybir.dt.float32
    bf16 = mybir.dt.bfloat16
    Alu = mybir.AluOpType
    Ident = mybir.ActivationFunctionType.Identity

    nc = bacc.Bacc("TRN2", target_bir_lowering=False, debug=False)
    x_d = nc.dram_tensor("x", [128, NKC * BPC], bf16, kind="ExternalInput").ap()
    w_d = nc.dram_tensor("w", [128, NKC * K], bf16, kind="ExternalInput").ap()
    b_d = nc.dram_tensor("b", [K, 1], f32, kind="ExternalInput").ap()
    o_d = nc.dram_tensor("out", [K, BPC], bf16, kind="ExternalOutput").ap()

    with tile.TileContext(nc) as tc:
        with (
            tc.tile_pool(name="const", bufs=1) as constp,
            tc.tile_pool(name="xs", bufs=len(DMA_KCS)) as xpool,
            tc.tile_pool(name="outs", bufs=4) as outp,
            tc.tile_pool(name="acc", bufs=NB, space="PSUM") as accp,
        ):
            QS = [nc.sync, nc.scalar]
            # w is split row-wise across the two hardware queues so both
            # rings carry the same byte load.
            bias = constp.tile([K, 1], f32)
            nc.sync.dma_start(bias[:], b_d[:])
            wsb = constp.tile([128, NKC * K], bf16)
            nc.sync.dma_start(wsb[0:64, :], w_d[0:64, :])
            nc.scalar.dma_start(wsb[64:128, :], w_d[64:128, :])

            # kc -> (tile, column offset within tile)
            xt, kc_slot = [], {}
            kc0 = 0
            for d, nkc in enumerate(DMA_KCS):
                t = xpool.tile([128, nkc * BPC], bf16, name=f"x{d}", tag="xs")
                QS[X_QUEUE[d]].dma_start(
                    t[:], x_d[:, kc0 * BPC:(kc0 + nkc) * BPC])
                xt.append(t)
                for j in range(nkc):
                    kc_slot[kc0 + j] = (t, j * BPC)
                kc0 += nkc

            accs = [accp.tile([K, BBLK], f32, name=f"acc{i}", tag="acc")
                    for i in range(NB)]
            for kc in range(NKC):
                t, off = kc_slot[kc]
                for bb in range(NB):
                    c0 = off + bb * BBLK
                    nc.tensor.matmul(
                        accs[bb][:],
                        wsb[:, kc * K:(kc + 1) * K],
                        t[:, c0:c0 + BBLK],
                        start=(kc == 0),
                        stop=(kc == NKC - 1),
                    )

            for bb in range(NB):
                # bias add on the scalar engine, LeakyReLU on vector: the two
                # blocks pipeline across engines at the tail.
                t1 = outp.tile([K, BBLK], f32)
                nc.scalar.activation(t1[:], accs[bb][:], Ident, bias=bias[:])
                ot = outp.tile([K, BBLK], bf16)
                nc.vector.scalar_tensor_tensor(
                    ot[:], t1[:], NEG_SLOPE, t1[:], op0=Alu.mult, op1=Alu.max)
                nc.sync.dma_start(o_d[:, bb * BBLK:(bb + 1) * BBLK], ot[:])
    nc.compile()
    return nc


def _prep_inputs(x, conv_w, conv_b):
    import ml_dtypes

    M = _dwt_matrix()  # [64, 84]
    # W_eff[(t,h,w), k] = sum_c M[t,c] conv_w[k,c,h,w]
    w_eff = np.einsum("tc,kchw->thwk", M, np.asarray(conv_w, dtype=np.float64))
    w2 = np.ascontiguousarray(w_eff.reshape(F, K)).astype(np.float32)
    # SBUF layout: wprep[p, kc*K + n] = w2[kc*128 + p, n]
    wprep = np.ascontiguousarray(
        w2.reshape(NKC, 128, K).transpose(1, 0, 2).reshape(128, NKC * K)
    ).astype(ml_dtypes.bfloat16)
    bias = np.ascontiguousarray(
        np.asarray(conv_b, dtype=np.float32).reshape(K, 1))
    # xprep[c, p, kc*BPC + b] = X[c*BPC + b, kc*128 + p], in bf16.
    xb = np.asarray(x).reshape(B, F).astype(ml_dtypes.bfloat16)
    xprep = np.ascontiguousarray(
        xb.reshape(NCORES, BPC, NKC, 128).transpose(0, 3, 2, 1)
    ).reshape(NCORES, 128, NKC * BPC)
    return xprep, wprep, bias


def _make_in_maps(x, conv_w, conv_b):
    xprep, wprep, bias = _prep_inputs(x, conv_w, conv_b)
    return [
        {"x": xprep[c], "w": wprep, "b": bias}
        for c in range(NCORES)
    ]


def kernel(x, conv_w, conv_b):
    from concourse.bass_utils import run_bass_kernel_spmd

    in_maps = _make_in_maps(x, conv_w, conv_b)
    nc = _build_bass()
    res = run_bass_kernel_spmd(nc, in_maps, list(range(NCORES)))
    out = np.concatenate(
        [np.asarray(r["out"]).astype(np.float32).T for r in res.results], axis=0)
    return np.ascontiguousarray(out, dtype=np.float32)


# revision 24
# speedup vs baseline: 1.0899x; 1.0899x over previous
"""Trainium2 kernel for nn_DWT_Features.

The reference applies a 3-level db4 DWT along the time axis of every
(batch, pixel) signal, then contracts the coefficients with a full-volume
conv kernel and applies LeakyReLU.  The DWT is a linear map sig[64] ->
coeffs[84], so the whole network collapses to a single GEMM:

    out = leaky_relu(X @ W_eff + b),  X: [B, 4096], W_eff: [4096, 64]

where W_eff[(t,h,w), k] = sum_c M[t, c] * conv_w[k, c, h, w] and M is the
64x84 DWT matrix (computed here in numpy, folded on host - O(22M) flops).

Sharding: pure data parallel, batch split across 8 cores (1024 rows each).

Device kernel design:
  - X is pre-transposed AND cast to bf16 on the host into the layout
    xprep[p, kc*1024 + b] = X[b, kc*128 + p]  (p: contraction partition,
    kc: one of 32 contraction chunks, b: batch column).  This removes all
    on-chip transposes (which dominated the first kernel: 256 tensor-engine
    transpose matmuls + PSUM->SBUF copies) and halves HBM traffic.
  - x streams in via [128, cols] DMAs alternating between the two hardware
    DGE queues (sync/scalar); w split row-wise across both queues.
  - GEMM: for each kc, two matmuls (batch blocks of 512) accumulate
    acc[64, 512] += W_kc.T @ X_kc in two PSUM banks; bf16 streams at
    1 col/cycle (216 ns per 512-col matmul, LDWEIGHTS hidden by the PE's
    reorder window), so the ~14 us of tensor work rides just behind the
    ~23 us wire-limited DMA stream (8.9 MB/core at ~380 GB/s).
  - epilogue: bias add on the scalar engine (ACTIVATE Identity + bias),
    LeakyReLU on vector, bf16 out [K, BPC]; host transposes/casts back.
"""

import sys

import numpy as np

if "/opt/trn_rl_repo" not in sys.path:
    sys.path.insert(0, "/opt/trn_rl_repo")

B, T, HW, K = 8192, 64, 8, 64
NCORES = 8
BPC = B // NCORES  # 1024 batch rows per core
F = T * HW * HW  # 4096 contracted features
NEG_SLOPE = 0.001
FILT_LEN = 8
NKC = F // 128  # 32 contraction chunks of 128
BBLK = 512  # batch columns per PSUM accumulator
NB = BPC // BBLK  # 2 batch blocks
# x DMA chunking (units of kc = BPC columns = 2 KiB/partition of bf16).
# Full-partition [128, cols] DMAs spread over the two hardware DGE queues
# (sync/scalar).  4-kc chunks (8 KiB descriptors) halve the descriptor
# count: all dynamic queues are dispatched by DMA engine 79, so descriptor
# bookkeeping makes it the stream straggler — fewer descriptors shrink its
# lag.  2-kc chunks at the head start the matmul stream earlier and at the
# tail shrink the last-chunk-landing to last-matmul gap.  Ten chunks per
# queue-pair keep both 8-deep DGE rings stall-free.
DMA_KCS = [2, 2, 4, 4, 4, 4, 4, 4, 2, 2]
X_QUEUE = [0, 1, 0, 1, 0, 1, 0, 1, 0, 1]  # 0=sync, 1=scalar
assert sum(DMA_KCS) == NKC

DB4_LO = np.array(
    [-0.010597401784997278, 0.032883011666982945, 0.030841381835986965,
     -0.18703481171888114, -0.02798376941698385, 0.6308807679295904,
     0.7148465705525415, 0.23037781330885523], dtype=np.float64)
DB4_HI = np.array(
    [-0.23037781330885523, 0.7148465705525415, -0.6308807679295904,
     -0.02798376941698385, 0.18703481171888114, 0.030841381835986965,
     0.032883011666982945, -0.010597401784997278], dtype=np.float64)


def _afb1d(x):
    # numpy mirror of the reference: reflect pad, correlate with reversed
    # filters, stride 2.  x: [N, n] float64.
    n = x.shape[-1]
    out = (n + FILT_LEN - 1) // 2
    p = 2 * (out - 1) - n + FILT_LEN
    xp = np.pad(x, ((0, 0), (p // 2, (p + 1) // 2)), mode="reflect")
    idx = 2 * np.arange(out)[:, None] + np.arange(FILT_LEN)[None, :]
    win = xp[:, idx]  # [N, out, 8]
    return win @ DB4_LO[::-1], win @ DB4_HI[::-1]


def _dwt_matrix():
    # M [64, 84] with coeffs = sig @ M (image of the identity basis).
    lo, his = np.eye(T, dtype=np.float64), []
    for _ in range(3):
        lo, hi = _afb1d(lo)
        his.append(hi)
    return np.concatenate([lo] + his, axis=-1)


def _build_bass():
    import concourse.bacc as bacc
    import concourse.mybir as mybir
    import concourse.tile as tile

    f32 = mybir.dt.float32
    bf16 = mybir.dt.bfloat16
    Alu = mybir.AluOpType
    Ident = mybir.ActivationFunctionType.Identity

    nc = bacc.Bacc("TRN2", target_bir_lowering=False, debug=False)
    x_d = nc.dram_tensor("x", [128, NKC * BPC], bf16, kind="ExternalInput").ap()
    w_d = nc.dram_tensor("w", [128, NKC * K], bf16, kind="ExternalInput").ap()
    b_d = nc.dram_tensor("b", [K, 1], f32, kind="ExternalInput").ap()
    o_d = nc.dram_tensor("out", [K, BPC], bf16, kind="ExternalOutput").ap()

    with tile.TileContext(nc) as tc:
        with (
            tc.tile_pool(name="const", bufs=1) as constp,
            tc.tile_pool(name="xs", bufs=len(DMA_KCS)) as xpool,
            tc.tile_pool(name="outs", bufs=4) as outp,
            tc.tile_pool(name="acc", bufs=NB, space="PSUM") as accp,
        ):
            QS = [nc.sync, nc.scalar]
            # w is split row-wise across the two hardware queues so both
            # rings carry the same byte load.
            bias = constp.tile([K, 1], f32)
            nc.sync.dma_start(bias[:], b_d[:])
            wsb = constp.tile([128, NKC * K], bf16)
            nc.sync.dma_start(wsb[0:64, :], w_d[0:64, :])
            nc.scalar.dma_start(wsb[64:128, :], w_d[64:128, :])

            # kc -> (tile, column offset within tile)
            xt, kc_slot = [], {}
            kc0 = 0
            for d, nkc in enumerate(DMA_KCS):
                t = xpool.tile([128, nkc * BPC], bf16, name=f"x{d}", tag="xs")
                QS[X_QUEUE[d]].dma_start(
                    t[:], x_d[:, kc0 * BPC:(kc0 + nkc) * BPC])
                xt.append(t)
                for j in range(nkc):
                    kc_slot[kc0 + j] = (t, j * BPC)
                kc0 += nkc

            accs = [accp.tile([K, BBLK], f32, name=f"acc{i}", tag="acc")
                    for i in range(NB)]
            for kc in range(NKC):
                t, off = kc_slot[kc]
                for bb in range(NB):
                    c0 = off + bb * BBLK
                    nc.tensor.matmul(
                        accs[bb][:],
                        wsb[:, kc * K:(kc + 1) * K],
                        t[:, c0:c0 + BBLK],
                        start=(kc == 0),
                        stop=(kc == NKC - 1),
                    )

            for bb in range(NB):
                # bias add on the scalar engine, LeakyReLU on vector: the two
                # blocks pipeline across engines at the tail.
                t1 = outp.tile([K, BBLK], f32)
                nc.scalar.activation(t1[:], accs[bb][:], Ident, bias=bias[:])
                ot = outp.tile([K, BBLK], bf16)
                nc.vector.scalar_tensor_tensor(
                    ot[:], t1[:], NEG_SLOPE, t1[:], op0=Alu.mult, op1=Alu.max)
                nc.sync.dma_start(o_d[:, bb * BBLK:(bb + 1) * BBLK], ot[:])
    nc.compile()
    return nc


def _prep_inputs(x, conv_w, conv_b):
    import ml_dtypes

    M = _dwt_matrix()  # [64, 84]
    # W_eff[(t,h,w), k] = sum_c M[t,c] conv_w[k,c,h,w]
    w_eff = np.einsum("tc,kchw->thwk", M, np.asarray(conv_w, dtype=np.float64))
    w2 = np.ascontiguousarray(w_eff.reshape(F, K)).astype(np.float32)
    # SBUF layout: wprep[p, kc*K + n] = w2[kc*128 + p, n]
    wprep = np.ascontiguousarray(
        w2.reshape(NKC, 128, K).transpose(1, 0, 2).reshape(128, NKC * K)
    ).astype(ml_dtypes.bfloat16)
    bias = np.ascontiguousarray(
        np.asarray(conv_b, dtype=np.float32).reshape(K, 1))
    # xprep[c, p, kc*BPC + b] = X[c*BPC + b, kc*128 + p], in bf16.
    xb = np.asarray(x).reshape(B, F).astype(ml_dtypes.bfloat16)
    xprep = np.ascontiguousarray(
        xb.reshape(NCORES, BPC, NKC, 128).transpose(0, 3, 2, 1)
    ).reshape(NCORES, 128, NKC * BPC)
    return xprep, wprep, bias


def _make_in_maps(x, conv_w, conv_b):
    xprep, wprep, bias = _prep_inputs(x, conv_w, conv_b)
    return [
        {"x": xprep[c], "w": wprep, "b": bias}
        for c in range(NCORES)
    ]


def kernel(x, conv_w, conv_b):
    from concourse.bass_utils import run_bass_kernel_spmd

    in_maps = _make_in_maps(x, conv_w, conv_b)
    nc = _build_bass()
    res = run_bass_kernel_spmd(nc, in_maps, list(range(NCORES)))
    out = np.concatenate(
        [np.asarray(r["out"]).astype(np.float32).T for r in res.results], axis=0)
    return np.ascontiguousarray(out, dtype=np.float32)
